# revision 8
# baseline (speedup 1.0000x reference)
"""MoE routing kernel (top-2 of 32 experts, dense-mix form) for 8 TRN2 cores.

out = sum_e mix_w[:, e] * (x @ W_e) + mix_b @ expert_biases, with mix_w / mix_b
the dense top-2 softmax mixtures from the two routers. Experts sharded
4-per-core; each core emits a bf16 partial; host sums the 8 partials.

Key design points (HW-measured on trn2, steady-state per-iteration):
- The kernel is weight-DMA-bound (~8.5 MB/core/iter). The weight stream is
  spread across the SWDGE (gpsimd) queue and both HWDGE rings (SP + ACT):
  a single ring measured ~40% slower end-to-end than the spread.
- Router logits stay fp32 end-to-end: top-2 selection is discontinuous, and
  bf16 logits flip ~2% of the picks, blowing the 2e-2 error gate.
- The two leading k-tiles of every expert ride as fp8e4m3 pre-scaled by 64
  (x^T for those k-tiles is scaled by 1/64 on chip, so products are exact);
  measured rel_l2 1.37e-2 vs the 2e-2 gate, and 12.5% less weight traffic.
- Expert 3's tail chunks ride HWDGE only (SWDGE descriptor-gen backlog must
  not delay them); its final half folds and stores in two 256-col pieces.
- Dummy warm-up matmuls release the PE HAM clock-gate before weights land.
"""

import sys

if "/opt/trn_rl_repo" not in sys.path:
    sys.path.insert(0, "/opt/trn_rl_repo")

from contextlib import ExitStack

import ml_dtypes
import numpy as np

import concourse.bacc as bacc
import concourse.tile as tile
from concourse import mybir
from concourse.bass_utils import run_bass_kernel_spmd
from concourse.masks import make_identity

B = 128        # batch
D = 1024       # in = out features
E = 32         # experts
NCORES = 8
EPC = E // NCORES   # experts per core
KT = D // 128       # k-tiles of 128 along contraction dim
HD = 512            # psum-bank-sized output chunk
GRP = B + 2 * E     # per-k group in xrw: [x^T (128) | routers (64)], fp32
AUXB = 2 * EPC + D  # bf16 aux cols: sel [64, 8] ++ biases [4, 8:8+D]
NF8 = 2             # leading k-tiles stored as fp8e4m3 (64x pre-scaled)

F32 = mybir.dt.float32
BF16 = mybir.dt.bfloat16
ALU = mybir.AluOpType
ACTF = mybir.ActivationFunctionType


def _ctile(pool, name, shape, dtype):
    # unique tag => dedicated slot, never rotated/reused
    return pool.tile(shape, dtype, name=name, tag=name)


def build_program(reps=1, channels=("gpsimd", "sync", "scalar"), nwarm=8,
                  wbufs=EPC, nf8=NF8, pin_e3="h1", ckt=6):
    nc = bacc.Bacc("TRN2")
    F8 = mybir.dt.float8e4
    KB = KT - nf8  # bf16 k-tiles per expert

    # fp32 router block: top-2 selection is discontinuous, so logits must be
    # computed at the reference's precision (bf16 logits flip ~2% of top-2
    # picks and blow the error gate)
    xrw_d = nc.dram_tensor("xrw", [128, KT, GRP], F32, kind="ExternalInput")
    auxb_d = nc.dram_tensor("auxb", [2 * E, AUXB], BF16, kind="ExternalInput")
    # k-tiles 0:nf8 ride as fp8e4m3 scaled by 64 (x^T those k-tiles are
    # pre-scaled by 1/64 on chip, so products come out exact); rest bf16
    wloc_d = nc.dram_tensor("wloc", [EPC, 128, KB, D], BF16, kind="ExternalInput")
    if nf8:
        wf8_d = nc.dram_tensor("wf8", [EPC, 128, nf8, D], F8, kind="ExternalInput")
    out_d = nc.dram_tensor("out", [B, D], BF16, kind="ExternalOutput")

    with ExitStack() as ctx:
        tc = ctx.enter_context(tile.TileContext(nc))
        const = ctx.enter_context(tc.tile_pool(name="const", bufs=1))
        xrpool = ctx.enter_context(tc.tile_pool(name="xrp", bufs=2))
        wpool = ctx.enter_context(tc.tile_pool(name="wts", bufs=wbufs))
        pp_a = ctx.enter_context(tc.tile_pool(name="pa", bufs=1, space="PSUM"))
        pp_t = ctx.enter_context(tc.tile_pool(name="pt", bufs=1, space="PSUM"))
        pp_e = ctx.enter_context(tc.tile_pool(name="pe", bufs=3, space="PSUM"))

        # PE warm-up fodder: zeroed bf16 tile, matmul'd before real work so
        # the HAM clock-gate is released by the time weights arrive
        wrm = _ctile(const, "wrm", [128, HD], BF16)
        if nwarm:
            nc.gpsimd.memset(wrm[:], 0.0)

        ident = _ctile(const, "ident", [128, 128], F32)
        make_identity(nc, ident[:])

        # independent DMA channels; rotate per chunk
        def chan(i):
            return getattr(nc, channels[i % len(channels)])

        for _ in range(reps):
            xrw = xrpool.tile([128, KT, GRP], F32, name="xrw")
            nc.sync.dma_start(xrw[:], xrw_d[:])
            auxb = xrpool.tile([2 * E, AUXB], BF16, name="auxb")
            nc.scalar.dma_start(auxb[:], auxb_d[:])

            # bf16 cast of x^T for the expert matmuls (router stays fp32);
            # the fp8 k-tiles get x/64 to cancel the 64x weight pre-scale
            xtb = xrpool.tile([128, KT, B], BF16, name="xtb")
            if nf8:
                nc.vector.tensor_scalar_mul(
                    xtb[:, 0:nf8, :], xrw[:, 0:nf8, 0:B], 1.0 / 64.0
                )
                nc.vector.tensor_copy(xtb[:, nf8:, :], xrw[:, nf8:, 0:B])
            else:
                nc.vector.tensor_copy(xtb[:], xrw[:, :, 0:B])

            def xt(k):
                return xtb[:, k, :]

            def rw(k):
                return xrw[:, k, B:GRP]

            sel = auxb[:, 0 : 2 * EPC]
            bscl = auxb[0:EPC, 2 * EPC : 2 * EPC + D]

            # ---- weight stream: per expert the fp8 block first (consumed
            # first by the k-loop), then bf16 half-K chunks; expert 3 in
            # column-half chunks (shorter tail); rotate channels per chunk
            wts = [wpool.tile([128, KB, D], BF16, name="w") for _ in range(EPC)]
            wts8 = [
                wpool.tile([128, nf8, D], mybir.dt.float8e4, name="w8")
                for _ in range(EPC)
            ] if nf8 else []
            ci = 0
            el = EPC - 1
            kranges = [(j, min(j + ckt, KB)) for j in range(0, KB, ckt)]
            for e in range(EPC - 1):
                if nf8:
                    chan(ci).dma_start(wts8[e][:, 0:nf8, :], wf8_d[e, :, :, :])
                    ci += 1
                for j0, j1 in kranges:
                    chan(ci).dma_start(
                        wts[e][:, j0:j1, :], wloc_d[e, :, j0:j1, :]
                    )
                    ci += 1
            # last expert's tail chunks ride HWDGE only: SWDGE descriptor-gen
            # backlog must not delay them (they bound the kernel tail).
            # pin_e3="all" pins both halves; "h1" only the final half.
            h1ch = (nc.sync, nc.scalar)
            hj = 0

            def ch(h):
                nonlocal ci, hj
                if pin_e3 == "all" or h == 1:
                    c = h1ch[hj % 2]
                    hj += 1
                else:
                    c = chan(ci)
                    ci += 1
                return c

            for h in range(2):
                hs, he = h * HD, (h + 1) * HD
                if nf8:
                    ch(h).dma_start(
                        wts8[el][:, 0:nf8, hs:he], wf8_d[el, :, :, hs:he]
                    )
                for j0, j1 in kranges:
                    ch(h).dma_start(
                        wts[el][:, j0:j1, hs:he],
                        wloc_d[el, :, j0:j1, hs:he],
                    )

            def wslice(e, k, cs, ce):
                if k < nf8:
                    return wts8[e][:, k, cs:ce]
                return wts[e][:, k - nf8, cs:ce]

            # ---- router logits: [B, 64] = x @ [router_w | bias_router_w] ----
            # nwarm dummy matmuls first: no input deps, so PE starts at t~0
            # and the HAM clock-gate is released before real work arrives.
            # They write the same PSUM bank; the k==0 router matmul's
            # start=True clears it, so the results never leak out.
            pl = pp_a.tile([B, HD], F32, name="pa")
            for i in range(nwarm):
                nc.tensor.matmul(
                    pl[:], wrm[:, 0:128], wrm[:],
                    start=(i == 0), stop=(i == nwarm - 1),
                )
            for k in range(KT):
                nc.tensor.matmul(
                    pl[:, 0 : 2 * E], xrw[:, k, 0:B], rw(k),
                    start=(k == 0), stop=(k == KT - 1),
                )
            logits = _ctile(const, "logits", [B, 2 * E], F32)
            nc.vector.tensor_copy(logits[:], pl[:, 0 : 2 * E])

            # expert 0 matmuls queue on PE ahead of the mix chain (PE is
            # in-order; DVE runs the softmax concurrently)
            pe0 = pp_e.tile([B, 2, HD], F32, name="pe")
            for k in range(KT):
                for c in range(2):
                    nc.tensor.matmul(
                        pe0[:, c, :], xt(k), wslice(0, k, c * HD, (c + 1) * HD),
                        start=(k == 0), stop=(k == KT - 1),
                    )

            # ---- top-2 + softmax per half -> dense mix coeffs [B, 64] ----
            mix_comb = _ctile(const, "mix_comb", [B, 2 * E], F32)
            for h in range(2):
                lh = logits[:, h * E : (h + 1) * E]
                mx1 = _ctile(const, f"mx1_{h}", [B, 1], F32)
                nc.vector.tensor_reduce(mx1[:], lh, axis=mybir.AxisListType.X, op=ALU.max)
                m1 = _ctile(const, f"m1_{h}", [B, E], F32)
                nc.vector.tensor_scalar(m1[:], lh, mx1[:], None, op0=ALU.is_ge)
                msk = _ctile(const, f"msk_{h}", [B, E], F32)
                nc.vector.scalar_tensor_tensor(
                    msk[:], m1[:], -1e30, lh, op0=ALU.mult, op1=ALU.add
                )
                mx2 = _ctile(const, f"mx2_{h}", [B, 1], F32)
                nc.vector.tensor_reduce(mx2[:], msk[:], axis=mybir.AxisListType.X, op=ALU.max)
                m2 = _ctile(const, f"m2_{h}", [B, E], F32)
                nc.vector.tensor_scalar(m2[:], msk[:], mx2[:], None, op0=ALU.is_ge)
                dgap = _ctile(const, f"dgap_{h}", [B, 1], F32)
                nc.vector.tensor_sub(dgap[:], mx2[:], mx1[:])
                ed = _ctile(const, f"ed_{h}", [B, 1], F32)
                nc.scalar.activation(ed[:], dgap[:], ACTF.Exp)
                den = _ctile(const, f"den_{h}", [B, 1], F32)
                nc.vector.tensor_scalar_add(den[:], ed[:], 1.0)
                p1 = _ctile(const, f"p1_{h}", [B, 1], F32)
                nc.vector.reciprocal(p1[:], den[:])
                p2 = _ctile(const, f"p2_{h}", [B, 1], F32)
                nc.vector.tensor_mul(p2[:], ed[:], p1[:])
                t2 = _ctile(const, f"t2_{h}", [B, E], F32)
                nc.vector.tensor_scalar_mul(t2[:], m2[:], p2[:])
                nc.vector.scalar_tensor_tensor(
                    mix_comb[:, h * E : (h + 1) * E], m1[:], p1[:], t2[:],
                    op0=ALU.mult, op1=ALU.add,
                )

            # ---- [B, 64] -> [64, B] transpose; bf16 for the sel matmuls ----
            ptm = pp_t.tile([2 * E, B], F32, name="pt")
            nc.tensor.transpose(ptm[:], mix_comb[:], ident[:])
            mixT = _ctile(const, "mixT", [2 * E, B], BF16)
            nc.vector.tensor_copy(mixT[:], ptm[:])

            # this core's weight/bias coefficients [B, 4+4]
            pml = pp_a.tile([B, 2 * E], F32, name="pa")
            nc.tensor.matmul(
                pml[:, 0 : 2 * EPC], mixT[:], sel, start=True, stop=True
            )
            mix_loc = _ctile(const, "mix_loc", [B, 2 * EPC], F32)
            nc.vector.tensor_copy(mix_loc[:], pml[:, 0 : 2 * EPC])

            # bias-mix transposed [4, B]: sel_b^T @ mixT (reuses mixT)
            pbt = pp_t.tile([2 * E, B], F32, name="pt")
            nc.tensor.matmul(
                pbt[0:EPC, :], sel[:, EPC : 2 * EPC], mixT[:], start=True, stop=True
            )
            mixbT = _ctile(const, "mixbT", [EPC, B], BF16)
            nc.vector.tensor_copy(mixbT[:], pbt[0:EPC, :])

            # ---- local bias term: mixb_loc @ bscl -> [B, D] ----
            pb = pp_e.tile([B, 2, HD], F32, name="pe")
            for c in range(2):
                nc.tensor.matmul(
                    pb[:, c, :], mixbT[:], bscl[:, c * HD : (c + 1) * HD],
                    start=True, stop=True,
                )
            bias_sb = _ctile(const, "bias_sb", [B, D], F32)
            for c in range(2):
                nc.vector.tensor_copy(bias_sb[:, c * HD : (c + 1) * HD], pb[:, c, :])

            # ---- experts: acc_e = (x @ W_e) * mix_loc[:, e] + acc_{e-1} ----
            prev = bias_sb
            for e in range(EPC - 1):
                if e == 0:
                    pe = pe0
                else:
                    pe = pp_e.tile([B, 2, HD], F32, name="pe")
                    for k in range(KT):
                        for c in range(2):
                            nc.tensor.matmul(
                                pe[:, c, :], xt(k), wslice(e, k, c * HD, (c + 1) * HD),
                                start=(k == 0), stop=(k == KT - 1),
                            )
                acc = _ctile(const, f"acc{e}", [B, D], F32)
                for c in range(2):
                    nc.vector.scalar_tensor_tensor(
                        acc[:, c * HD : (c + 1) * HD], pe[:, c, :], mix_loc[:, e : e + 1],
                        prev[:, c * HD : (c + 1) * HD], op0=ALU.mult, op1=ALU.add,
                    )
                prev = acc

            # last expert: per-half compute -> fold -> output DMA
            pel = pp_e.tile([B, 2, HD], F32, name="pe")
            accl = _ctile(const, f"acc{el}", [B, D], BF16)
            for h in range(2):
                hs, he = h * HD, (h + 1) * HD
                for k in range(KT):
                    nc.tensor.matmul(
                        pel[:, h, :], xt(k), wslice(el, k, hs, he),
                        start=(k == 0), stop=(k == KT - 1),
                    )
                if h == 0:
                    nc.vector.scalar_tensor_tensor(
                        accl[:, hs:he], pel[:, h, :], mix_loc[:, el : el + 1],
                        prev[:, hs:he], op0=ALU.mult, op1=ALU.add,
                    )
                    nc.scalar.dma_start(out_d[:, hs:he], accl[:, hs:he])
                else:
                    # final half in two 256-col pieces: fold of piece 1
                    # overlaps the output DMA of piece 0
                    for q in range(2):
                        qs, qe = hs + q * (HD // 2), hs + (q + 1) * (HD // 2)
                        nc.vector.scalar_tensor_tensor(
                            accl[:, qs:qe], pel[:, h, qs - hs : qe - hs],
                            mix_loc[:, el : el + 1],
                            prev[:, qs:qe], op0=ALU.mult, op1=ALU.add,
                        )
                        eng = nc.sync if q == 1 else nc.scalar
                        eng.dma_start(out_d[:, qs:qe], accl[:, qs:qe])

    nc.finalize()
    return nc


def make_input_maps(x, router_w, bias_router_w, expert_weights, expert_biases,
                    nf8=NF8):
    bf = ml_dtypes.bfloat16
    xt = np.asarray(x, dtype=np.float32).T.reshape(KT, 128, B).transpose(1, 0, 2)
    rw2 = (
        np.concatenate(
            [np.asarray(router_w, np.float32), np.asarray(bias_router_w, np.float32)],
            axis=1,
        )
        .reshape(KT, 128, 2 * E)
        .transpose(1, 0, 2)
    )
    xrw = np.ascontiguousarray(
        np.concatenate([xt, rw2], axis=2), dtype=np.float32
    )  # [128, KT, 192]

    ew = np.asarray(expert_weights, np.float32)
    eb = np.asarray(expert_biases, np.float32)

    in_maps = []
    for c in range(NCORES):
        auxb = np.zeros((2 * E, AUXB), dtype=bf)
        selc = np.zeros((2 * E, 2 * EPC), dtype=np.float32)
        for j in range(EPC):
            selc[c * EPC + j, j] = 1.0
            selc[E + c * EPC + j, EPC + j] = 1.0
        auxb[:, 0 : 2 * EPC] = selc.astype(bf)
        auxb[0:EPC, 2 * EPC : 2 * EPC + D] = eb[c * EPC : (c + 1) * EPC].astype(bf)

        wall = (
            ew[c * EPC : (c + 1) * EPC]
            .reshape(EPC, KT, 128, D)
            .transpose(0, 2, 1, 3)
        )  # [EPC, 128, KT, D]
        wl = np.ascontiguousarray(wall[:, :, nf8:, :]).astype(bf)
        m = dict(xrw=xrw, auxb=auxb, wloc=wl)
        if nf8:
            m["wf8"] = np.ascontiguousarray(wall[:, :, 0:nf8, :] * 64.0).astype(
                ml_dtypes.float8_e4m3
            )
        in_maps.append(m)
    return in_maps


def kernel(x, router_w, bias_router_w, expert_weights, expert_biases, **bench_kwargs):
    in_maps = make_input_maps(x, router_w, bias_router_w, expert_weights, expert_biases)
    nc = build_program()
    res = run_bass_kernel_spmd(nc, in_maps, list(range(NCORES)), **bench_kwargs)
    out = np.zeros((B, D), dtype=np.float64)
    for r in res.results:
        out += r["out"].astype(np.float64)
    final = out.astype(np.float32)
    if bench_kwargs:
        kernel.last_result = res
    return final


# revision 11
# speedup vs baseline: 1.0016x; 1.0016x over previous
"""MoE routing kernel (top-2 of 32 experts, dense-mix form) for 8 TRN2 cores.

out = sum_e mix_w[:, e] * (x @ W_e) + mix_b @ expert_biases, with mix_w / mix_b
the dense top-2 softmax mixtures from the two routers. Experts sharded
4-per-core; each core emits a bf16 partial; host sums the 8 partials.

Key design points (HW-measured on trn2, steady-state per-iteration):
- The kernel is weight-DMA-bound (~8.5 MB/core/iter). The weight stream is
  spread across the SWDGE (gpsimd) queue and both HWDGE rings (SP + ACT):
  a single ring measured ~40% slower end-to-end than the spread.
- Router logits stay fp32 end-to-end: top-2 selection is discontinuous, and
  bf16 logits flip ~2% of the picks, blowing the 2e-2 error gate.
- The two leading k-tiles of every expert ride as fp8e4m3 pre-scaled by 64
  (x^T for those k-tiles is scaled by 1/64 on chip, so products are exact);
  measured rel_l2 1.37e-2 vs the 2e-2 gate, and 12.5% less weight traffic.
- Expert 3's tail chunks ride HWDGE only (SWDGE descriptor-gen backlog must
  not delay them); its final half folds and stores in two 256-col pieces.
- Dummy warm-up matmuls release the PE HAM clock-gate before weights land.
"""

import sys

if "/opt/trn_rl_repo" not in sys.path:
    sys.path.insert(0, "/opt/trn_rl_repo")

from contextlib import ExitStack

import ml_dtypes
import numpy as np

import concourse.bacc as bacc
import concourse.tile as tile
from concourse import mybir
from concourse.bass_utils import run_bass_kernel_spmd
from concourse.masks import make_identity

B = 128        # batch
D = 1024       # in = out features
E = 32         # experts
NCORES = 8
EPC = E // NCORES   # experts per core
KT = D // 128       # k-tiles of 128 along contraction dim
HD = 512            # psum-bank-sized output chunk
GRP = B + 2 * E     # per-k group in xrw: [x^T (128) | routers (64)], fp32
AUXB = 2 * EPC + D  # bf16 aux cols: sel [64, 8] ++ biases [4, 8:8+D]
NF8 = 2             # leading k-tiles stored as fp8e4m3 (64x pre-scaled)

F32 = mybir.dt.float32
BF16 = mybir.dt.bfloat16
ALU = mybir.AluOpType
ACTF = mybir.ActivationFunctionType


def _ctile(pool, name, shape, dtype):
    # unique tag => dedicated slot, never rotated/reused
    return pool.tile(shape, dtype, name=name, tag=name)


def build_program(reps=1, channels=("gpsimd", "sync", "scalar"), nwarm=8,
                  wbufs=EPC, nf8=NF8, pin_e3="h1", ckt=6, tailkt=4):
    nc = bacc.Bacc("TRN2")
    F8 = mybir.dt.float8e4
    KB = KT - nf8  # bf16 k-tiles per expert

    # fp32 router block: top-2 selection is discontinuous, so logits must be
    # computed at the reference's precision (bf16 logits flip ~2% of top-2
    # picks and blow the error gate)
    xrw_d = nc.dram_tensor("xrw", [128, KT, GRP], F32, kind="ExternalInput")
    auxb_d = nc.dram_tensor("auxb", [2 * E, AUXB], BF16, kind="ExternalInput")
    # k-tiles 0:nf8 ride as fp8e4m3 scaled by 64 (x^T those k-tiles are
    # pre-scaled by 1/64 on chip, so products come out exact); rest bf16
    wloc_d = nc.dram_tensor("wloc", [EPC, 128, KB, D], BF16, kind="ExternalInput")
    if nf8:
        wf8_d = nc.dram_tensor("wf8", [EPC, 128, nf8, D], F8, kind="ExternalInput")
    out_d = nc.dram_tensor("out", [B, D], BF16, kind="ExternalOutput")

    with ExitStack() as ctx:
        tc = ctx.enter_context(tile.TileContext(nc))
        const = ctx.enter_context(tc.tile_pool(name="const", bufs=1))
        xrpool = ctx.enter_context(tc.tile_pool(name="xrp", bufs=2))
        wpool = ctx.enter_context(tc.tile_pool(name="wts", bufs=wbufs))
        pp_a = ctx.enter_context(tc.tile_pool(name="pa", bufs=1, space="PSUM"))
        pp_t = ctx.enter_context(tc.tile_pool(name="pt", bufs=1, space="PSUM"))
        pp_e = ctx.enter_context(tc.tile_pool(name="pe", bufs=3, space="PSUM"))

        # PE warm-up fodder: zeroed bf16 tile, matmul'd before real work so
        # the HAM clock-gate is released by the time weights arrive
        wrm = _ctile(const, "wrm", [128, HD], BF16)
        if nwarm:
            nc.gpsimd.memset(wrm[:], 0.0)

        ident = _ctile(const, "ident", [128, 128], F32)
        make_identity(nc, ident[:])

        # independent DMA channels; rotate per chunk
        def chan(i):
            return getattr(nc, channels[i % len(channels)])

        for _ in range(reps):
            xrw = xrpool.tile([128, KT, GRP], F32, name="xrw")
            nc.sync.dma_start(xrw[:], xrw_d[:])
            auxb = xrpool.tile([2 * E, AUXB], BF16, name="auxb")
            nc.scalar.dma_start(auxb[:], auxb_d[:])

            # bf16 cast of x^T for the expert matmuls (router stays fp32);
            # the fp8 k-tiles get x/64 to cancel the 64x weight pre-scale
            xtb = xrpool.tile([128, KT, B], BF16, name="xtb")
            if nf8:
                nc.vector.tensor_scalar_mul(
                    xtb[:, 0:nf8, :], xrw[:, 0:nf8, 0:B], 1.0 / 64.0
                )
                nc.vector.tensor_copy(xtb[:, nf8:, :], xrw[:, nf8:, 0:B])
            else:
                nc.vector.tensor_copy(xtb[:], xrw[:, :, 0:B])

            def xt(k):
                return xtb[:, k, :]

            def rw(k):
                return xrw[:, k, B:GRP]

            sel = auxb[:, 0 : 2 * EPC]
            bscl = auxb[0:EPC, 2 * EPC : 2 * EPC + D]

            # ---- weight stream: per expert the fp8 block first (consumed
            # first by the k-loop), then bf16 half-K chunks; expert 3 in
            # column-half chunks (shorter tail); rotate channels per chunk
            wts = [wpool.tile([128, KB, D], BF16, name="w") for _ in range(EPC)]
            wts8 = [
                wpool.tile([128, nf8, D], mybir.dt.float8e4, name="w8")
                for _ in range(EPC)
            ] if nf8 else []
            ci = 0
            el = EPC - 1
            kranges = [(j, min(j + ckt, KB)) for j in range(0, KB, ckt)]
            for e in range(EPC - 1):
                if nf8:
                    chan(ci).dma_start(wts8[e][:, 0:nf8, :], wf8_d[e, :, :, :])
                    ci += 1
                for j0, j1 in kranges:
                    chan(ci).dma_start(
                        wts[e][:, j0:j1, :], wloc_d[e, :, j0:j1, :]
                    )
                    ci += 1
            # last expert's tail chunks ride HWDGE only: SWDGE descriptor-gen
            # backlog must not delay them (they bound the kernel tail).
            # pin_e3="all" pins both halves; "h1" only the final half.
            h1ch = (nc.sync, nc.scalar)
            hj = 0

            def ch(h):
                nonlocal ci, hj
                if pin_e3 == "all" or h == 1:
                    c = h1ch[hj % 2]
                    hj += 1
                else:
                    c = chan(ci)
                    ci += 1
                return c

            for h in range(2):
                hs, he = h * HD, (h + 1) * HD
                if nf8:
                    ch(h).dma_start(
                        wts8[el][:, 0:nf8, hs:he], wf8_d[el, :, :, hs:he]
                    )
                # the very last chunks can be smaller (tailkt): fewer
                # matmuls remain after the final weight byte lands
                kr = kranges
                if h == 1 and tailkt:
                    kr = [(j, min(j + tailkt, KB)) for j in range(0, KB, tailkt)]
                for j0, j1 in kr:
                    ch(h).dma_start(
                        wts[el][:, j0:j1, hs:he],
                        wloc_d[el, :, j0:j1, hs:he],
                    )

            def wslice(e, k, cs, ce):
                if k < nf8:
                    return wts8[e][:, k, cs:ce]
                return wts[e][:, k - nf8, cs:ce]

            # ---- router logits: [B, 64] = x @ [router_w | bias_router_w] ----
            # nwarm dummy matmuls first: no input deps, so PE starts at t~0
            # and the HAM clock-gate is released before real work arrives.
            # They write the same PSUM bank; the k==0 router matmul's
            # start=True clears it, so the results never leak out.
            pl = pp_a.tile([B, HD], F32, name="pa")
            for i in range(nwarm):
                nc.tensor.matmul(
                    pl[:], wrm[:, 0:128], wrm[:],
                    start=(i == 0), stop=(i == nwarm - 1),
                )
            for k in range(KT):
                nc.tensor.matmul(
                    pl[:, 0 : 2 * E], xrw[:, k, 0:B], rw(k),
                    start=(k == 0), stop=(k == KT - 1),
                )
            logits = _ctile(const, "logits", [B, 2 * E], F32)
            nc.vector.tensor_copy(logits[:], pl[:, 0 : 2 * E])

            # expert 0 matmuls queue on PE ahead of the mix chain (PE is
            # in-order; DVE runs the softmax concurrently)
            pe0 = pp_e.tile([B, 2, HD], F32, name="pe")
            for k in range(KT):
                for c in range(2):
                    nc.tensor.matmul(
                        pe0[:, c, :], xt(k), wslice(0, k, c * HD, (c + 1) * HD),
                        start=(k == 0), stop=(k == KT - 1),
                    )

            # ---- top-2 + softmax per half -> dense mix coeffs [B, 64] ----
            mix_comb = _ctile(const, "mix_comb", [B, 2 * E], F32)
            for h in range(2):
                lh = logits[:, h * E : (h + 1) * E]
                mx1 = _ctile(const, f"mx1_{h}", [B, 1], F32)
                nc.vector.tensor_reduce(mx1[:], lh, axis=mybir.AxisListType.X, op=ALU.max)
                m1 = _ctile(const, f"m1_{h}", [B, E], F32)
                nc.vector.tensor_scalar(m1[:], lh, mx1[:], None, op0=ALU.is_ge)
                msk = _ctile(const, f"msk_{h}", [B, E], F32)
                nc.vector.scalar_tensor_tensor(
                    msk[:], m1[:], -1e30, lh, op0=ALU.mult, op1=ALU.add
                )
                mx2 = _ctile(const, f"mx2_{h}", [B, 1], F32)
                nc.vector.tensor_reduce(mx2[:], msk[:], axis=mybir.AxisListType.X, op=ALU.max)
                m2 = _ctile(const, f"m2_{h}", [B, E], F32)
                nc.vector.tensor_scalar(m2[:], msk[:], mx2[:], None, op0=ALU.is_ge)
                dgap = _ctile(const, f"dgap_{h}", [B, 1], F32)
                nc.vector.tensor_sub(dgap[:], mx2[:], mx1[:])
                ed = _ctile(const, f"ed_{h}", [B, 1], F32)
                nc.scalar.activation(ed[:], dgap[:], ACTF.Exp)
                den = _ctile(const, f"den_{h}", [B, 1], F32)
                nc.vector.tensor_scalar_add(den[:], ed[:], 1.0)
                p1 = _ctile(const, f"p1_{h}", [B, 1], F32)
                nc.vector.reciprocal(p1[:], den[:])
                p2 = _ctile(const, f"p2_{h}", [B, 1], F32)
                nc.vector.tensor_mul(p2[:], ed[:], p1[:])
                t2 = _ctile(const, f"t2_{h}", [B, E], F32)
                nc.vector.tensor_scalar_mul(t2[:], m2[:], p2[:])
                nc.vector.scalar_tensor_tensor(
                    mix_comb[:, h * E : (h + 1) * E], m1[:], p1[:], t2[:],
                    op0=ALU.mult, op1=ALU.add,
                )

            # ---- [B, 64] -> [64, B] transpose; bf16 for the sel matmuls ----
            ptm = pp_t.tile([2 * E, B], F32, name="pt")
            nc.tensor.transpose(ptm[:], mix_comb[:], ident[:])
            mixT = _ctile(const, "mixT", [2 * E, B], BF16)
            nc.vector.tensor_copy(mixT[:], ptm[:])

            # this core's weight/bias coefficients [B, 4+4]
            pml = pp_a.tile([B, 2 * E], F32, name="pa")
            nc.tensor.matmul(
                pml[:, 0 : 2 * EPC], mixT[:], sel, start=True, stop=True
            )
            mix_loc = _ctile(const, "mix_loc", [B, 2 * EPC], F32)
            nc.vector.tensor_copy(mix_loc[:], pml[:, 0 : 2 * EPC])

            # bias-mix transposed [4, B]: sel_b^T @ mixT (reuses mixT)
            pbt = pp_t.tile([2 * E, B], F32, name="pt")
            nc.tensor.matmul(
                pbt[0:EPC, :], sel[:, EPC : 2 * EPC], mixT[:], start=True, stop=True
            )
            mixbT = _ctile(const, "mixbT", [EPC, B], BF16)
            nc.vector.tensor_copy(mixbT[:], pbt[0:EPC, :])

            # ---- local bias term: mixb_loc @ bscl -> [B, D] ----
            pb = pp_e.tile([B, 2, HD], F32, name="pe")
            for c in range(2):
                nc.tensor.matmul(
                    pb[:, c, :], mixbT[:], bscl[:, c * HD : (c + 1) * HD],
                    start=True, stop=True,
                )
            bias_sb = _ctile(const, "bias_sb", [B, D], F32)
            for c in range(2):
                nc.vector.tensor_copy(bias_sb[:, c * HD : (c + 1) * HD], pb[:, c, :])

            # ---- experts: acc_e = (x @ W_e) * mix_loc[:, e] + acc_{e-1} ----
            prev = bias_sb
            for e in range(EPC - 1):
                if e == 0:
                    pe = pe0
                else:
                    pe = pp_e.tile([B, 2, HD], F32, name="pe")
                    for k in range(KT):
                        for c in range(2):
                            nc.tensor.matmul(
                                pe[:, c, :], xt(k), wslice(e, k, c * HD, (c + 1) * HD),
                                start=(k == 0), stop=(k == KT - 1),
                            )
                acc = _ctile(const, f"acc{e}", [B, D], F32)
                for c in range(2):
                    nc.vector.scalar_tensor_tensor(
                        acc[:, c * HD : (c + 1) * HD], pe[:, c, :], mix_loc[:, e : e + 1],
                        prev[:, c * HD : (c + 1) * HD], op0=ALU.mult, op1=ALU.add,
                    )
                prev = acc

            # last expert: per-half compute -> fold -> output DMA
            pel = pp_e.tile([B, 2, HD], F32, name="pe")
            accl = _ctile(const, f"acc{el}", [B, D], BF16)
            for h in range(2):
                hs, he = h * HD, (h + 1) * HD
                for k in range(KT):
                    nc.tensor.matmul(
                        pel[:, h, :], xt(k), wslice(el, k, hs, he),
                        start=(k == 0), stop=(k == KT - 1),
                    )
                if h == 0:
                    nc.vector.scalar_tensor_tensor(
                        accl[:, hs:he], pel[:, h, :], mix_loc[:, el : el + 1],
                        prev[:, hs:he], op0=ALU.mult, op1=ALU.add,
                    )
                    nc.scalar.dma_start(out_d[:, hs:he], accl[:, hs:he])
                else:
                    # final half in two 256-col pieces: fold of piece 1
                    # overlaps the output DMA of piece 0
                    for q in range(2):
                        qs, qe = hs + q * (HD // 2), hs + (q + 1) * (HD // 2)
                        nc.vector.scalar_tensor_tensor(
                            accl[:, qs:qe], pel[:, h, qs - hs : qe - hs],
                            mix_loc[:, el : el + 1],
                            prev[:, qs:qe], op0=ALU.mult, op1=ALU.add,
                        )
                        eng = nc.sync if q == 1 else nc.scalar
                        eng.dma_start(out_d[:, qs:qe], accl[:, qs:qe])

    nc.finalize()
    return nc


def make_input_maps(x, router_w, bias_router_w, expert_weights, expert_biases,
                    nf8=NF8):
    bf = ml_dtypes.bfloat16
    xt = np.asarray(x, dtype=np.float32).T.reshape(KT, 128, B).transpose(1, 0, 2)
    rw2 = (
        np.concatenate(
            [np.asarray(router_w, np.float32), np.asarray(bias_router_w, np.float32)],
            axis=1,
        )
        .reshape(KT, 128, 2 * E)
        .transpose(1, 0, 2)
    )
    xrw = np.ascontiguousarray(
        np.concatenate([xt, rw2], axis=2), dtype=np.float32
    )  # [128, KT, 192]

    ew = np.asarray(expert_weights, np.float32)
    eb = np.asarray(expert_biases, np.float32)

    in_maps = []
    for c in range(NCORES):
        auxb = np.zeros((2 * E, AUXB), dtype=bf)
        selc = np.zeros((2 * E, 2 * EPC), dtype=np.float32)
        for j in range(EPC):
            selc[c * EPC + j, j] = 1.0
            selc[E + c * EPC + j, EPC + j] = 1.0
        auxb[:, 0 : 2 * EPC] = selc.astype(bf)
        auxb[0:EPC, 2 * EPC : 2 * EPC + D] = eb[c * EPC : (c + 1) * EPC].astype(bf)

        wall = (
            ew[c * EPC : (c + 1) * EPC]
            .reshape(EPC, KT, 128, D)
            .transpose(0, 2, 1, 3)
        )  # [EPC, 128, KT, D]
        wl = np.ascontiguousarray(wall[:, :, nf8:, :]).astype(bf)
        m = dict(xrw=xrw, auxb=auxb, wloc=wl)
        if nf8:
            m["wf8"] = np.ascontiguousarray(wall[:, :, 0:nf8, :] * 64.0).astype(
                ml_dtypes.float8_e4m3
            )
        in_maps.append(m)
    return in_maps


def kernel(x, router_w, bias_router_w, expert_weights, expert_biases, **bench_kwargs):
    in_maps = make_input_maps(x, router_w, bias_router_w, expert_weights, expert_biases)
    nc = build_program()
    res = run_bass_kernel_spmd(nc, in_maps, list(range(NCORES)), **bench_kwargs)
    out = np.zeros((B, D), dtype=np.float64)
    for r in res.results:
        out += r["out"].astype(np.float64)
    final = out.astype(np.float32)
    if bench_kwargs:
        kernel.last_result = res
    return final


# revision 17
# speedup vs baseline: 1.0016x; 1.0001x over previous
"""MoE routing kernel (top-2 of 32 experts, dense-mix form) for 8 TRN2 cores.

out = sum_e mix_w[:, e] * (x @ W_e) + mix_b @ expert_biases, with mix_w / mix_b
the dense top-2 softmax mixtures from the two routers. Experts sharded
4-per-core; each core emits a bf16 partial; host sums the 8 partials.

Key design points (HW-measured on trn2, steady-state per-iteration):
- The kernel is weight-DMA-bound (~8.5 MB/core/iter). The weight stream is
  spread across the SWDGE (gpsimd) queue and both HWDGE rings (SP + ACT):
  a single ring measured ~40% slower end-to-end than the spread.
- Router logits stay fp32 end-to-end: top-2 selection is discontinuous, and
  bf16 logits flip ~2% of the picks, blowing the 2e-2 error gate.
- The two leading k-tiles of every expert ride as fp8e4m3 pre-scaled by 64
  (x^T for those k-tiles is scaled by 1/64 on chip, so products are exact);
  measured rel_l2 1.37e-2 vs the 2e-2 gate, and 12.5% less weight traffic.
- Expert 3's tail chunks ride HWDGE only (SWDGE descriptor-gen backlog must
  not delay them); its final half folds and stores in two 256-col pieces.
- Dummy warm-up matmuls release the PE HAM clock-gate before weights land.
"""

import sys

if "/opt/trn_rl_repo" not in sys.path:
    sys.path.insert(0, "/opt/trn_rl_repo")

from contextlib import ExitStack

import ml_dtypes
import numpy as np

import concourse.bacc as bacc
import concourse.tile as tile
from concourse import mybir
from concourse.bass_utils import run_bass_kernel_spmd
from concourse.masks import make_identity

B = 128        # batch
D = 1024       # in = out features
E = 32         # experts
NCORES = 8
EPC = E // NCORES   # experts per core
KT = D // 128       # k-tiles of 128 along contraction dim
HD = 512            # psum-bank-sized output chunk
GRP = B + 2 * E     # per-k group in xrw: [x^T (128) | routers (64)], fp32
NF8 = 2             # leading k-tiles stored as fp8e4m3 (64x pre-scaled)

F32 = mybir.dt.float32
BF16 = mybir.dt.bfloat16
ALU = mybir.AluOpType
ACTF = mybir.ActivationFunctionType


def _ctile(pool, name, shape, dtype):
    # unique tag => dedicated slot, never rotated/reused
    return pool.tile(shape, dtype, name=name, tag=name)


def build_program(reps=1, channels=("gpsimd", "sync", "scalar"), nwarm=8,
                  wbufs=EPC, nf8=NF8, pin_e3="h1", ckt=6, tailkt=4):
    nc = bacc.Bacc("TRN2")
    F8 = mybir.dt.float8e4
    KB = KT - nf8  # bf16 k-tiles per expert

    # fp32 router block: top-2 selection is discontinuous, so logits must be
    # computed at the reference's precision (bf16 logits flip ~2% of top-2
    # picks and blow the error gate)
    xrw_d = nc.dram_tensor("xrw", [128, KT, GRP], F32, kind="ExternalInput")
    sel_d = nc.dram_tensor("selt", [2 * E, 2 * EPC], BF16, kind="ExternalInput")
    bscl_d = nc.dram_tensor("bsclt", [EPC, D], BF16, kind="ExternalInput")
    # k-tiles 0:nf8 ride as fp8e4m3 scaled by 64 (x^T those k-tiles are
    # pre-scaled by 1/64 on chip, so products come out exact); rest bf16
    wloc_d = nc.dram_tensor("wloc", [EPC, 128, KB, D], BF16, kind="ExternalInput")
    if nf8:
        wf8_d = nc.dram_tensor("wf8", [EPC, 128, nf8, D], F8, kind="ExternalInput")
    out_d = nc.dram_tensor("out", [B, D], BF16, kind="ExternalOutput")

    with ExitStack() as ctx:
        tc = ctx.enter_context(tile.TileContext(nc))
        const = ctx.enter_context(tc.tile_pool(name="const", bufs=1))
        xrpool = ctx.enter_context(tc.tile_pool(name="xrp", bufs=2))
        wpool = ctx.enter_context(tc.tile_pool(name="wts", bufs=wbufs))
        pp_a = ctx.enter_context(tc.tile_pool(name="pa", bufs=1, space="PSUM"))
        pp_t = ctx.enter_context(tc.tile_pool(name="pt", bufs=1, space="PSUM"))
        pp_e = ctx.enter_context(tc.tile_pool(name="pe", bufs=3, space="PSUM"))

        # PE warm-up fodder: zeroed bf16 tile, matmul'd before real work so
        # the HAM clock-gate is released by the time weights arrive
        wrm = _ctile(const, "wrm", [128, HD], BF16)
        if nwarm:
            nc.gpsimd.memset(wrm[:], 0.0)

        ident = _ctile(const, "ident", [128, 128], F32)
        make_identity(nc, ident[:])

        # independent DMA channels; rotate per chunk
        def chan(i):
            return getattr(nc, channels[i % len(channels)])

        for _ in range(reps):
            xrw = xrpool.tile([128, KT, GRP], F32, name="xrw")
            nc.sync.dma_start(xrw[:], xrw_d[:])
            selt = xrpool.tile([2 * E, 2 * EPC], BF16, name="selt")
            nc.scalar.dma_start(selt[:], sel_d[:])
            bsclt = xrpool.tile([EPC, D], BF16, name="bsclt")
            nc.scalar.dma_start(bsclt[:], bscl_d[:])

            # bf16 cast of x^T for the expert matmuls (router stays fp32);
            # the fp8 k-tiles get x/64 to cancel the 64x weight pre-scale
            xtb = xrpool.tile([128, KT, B], BF16, name="xtb")
            if nf8:
                nc.vector.tensor_scalar_mul(
                    xtb[:, 0:nf8, :], xrw[:, 0:nf8, 0:B], 1.0 / 64.0
                )
                nc.vector.tensor_copy(xtb[:, nf8:, :], xrw[:, nf8:, 0:B])
            else:
                nc.vector.tensor_copy(xtb[:], xrw[:, :, 0:B])

            def xt(k):
                return xtb[:, k, :]

            def rw(k):
                return xrw[:, k, B:GRP]

            sel = selt[:]
            bscl = bsclt[:]

            # ---- weight stream: per expert the fp8 block first (consumed
            # first by the k-loop), then bf16 half-K chunks; expert 3 in
            # column-half chunks (shorter tail); rotate channels per chunk
            wts = [wpool.tile([128, KB, D], BF16, name="w") for _ in range(EPC)]
            wts8 = [
                wpool.tile([128, nf8, D], mybir.dt.float8e4, name="w8")
                for _ in range(EPC)
            ] if nf8 else []
            ci = 0
            el = EPC - 1
            kranges = [(j, min(j + ckt, KB)) for j in range(0, KB, ckt)]
            for e in range(EPC - 1):
                if nf8:
                    chan(ci).dma_start(wts8[e][:, 0:nf8, :], wf8_d[e, :, :, :])
                    ci += 1
                for j0, j1 in kranges:
                    chan(ci).dma_start(
                        wts[e][:, j0:j1, :], wloc_d[e, :, j0:j1, :]
                    )
                    ci += 1
            # last expert's tail chunks ride HWDGE only: SWDGE descriptor-gen
            # backlog must not delay them (they bound the kernel tail).
            # pin_e3="all" pins both halves; "h1" only the final half.
            h1ch = (nc.sync, nc.scalar)
            hj = 0

            def ch(h):
                nonlocal ci, hj
                if pin_e3 == "all" or h == 1:
                    c = h1ch[hj % 2]
                    hj += 1
                else:
                    c = chan(ci)
                    ci += 1
                return c

            for h in range(2):
                hs, he = h * HD, (h + 1) * HD
                if nf8:
                    ch(h).dma_start(
                        wts8[el][:, 0:nf8, hs:he], wf8_d[el, :, :, hs:he]
                    )
                # the very last chunks can be smaller (tailkt): fewer
                # matmuls remain after the final weight byte lands
                kr = kranges
                if h == 1 and tailkt:
                    kr = [(j, min(j + tailkt, KB)) for j in range(0, KB, tailkt)]
                for j0, j1 in kr:
                    ch(h).dma_start(
                        wts[el][:, j0:j1, hs:he],
                        wloc_d[el, :, j0:j1, hs:he],
                    )

            def wslice(e, k, cs, ce):
                if k < nf8:
                    return wts8[e][:, k, cs:ce]
                return wts[e][:, k - nf8, cs:ce]

            # ---- router logits: [B, 64] = x @ [router_w | bias_router_w] ----
            # nwarm dummy matmuls first: no input deps, so PE starts at t~0
            # and the HAM clock-gate is released before real work arrives.
            # They write the same PSUM bank; the k==0 router matmul's
            # start=True clears it, so the results never leak out.
            pl = pp_a.tile([B, HD], F32, name="pa")
            for i in range(nwarm):
                nc.tensor.matmul(
                    pl[:], wrm[:, 0:128], wrm[:],
                    start=(i == 0), stop=(i == nwarm - 1),
                )
            for k in range(KT):
                nc.tensor.matmul(
                    pl[:, 0 : 2 * E], xrw[:, k, 0:B], rw(k),
                    start=(k == 0), stop=(k == KT - 1),
                )
            logits = _ctile(const, "logits", [B, 2 * E], F32)
            nc.vector.tensor_copy(logits[:], pl[:, 0 : 2 * E])

            # expert 0 matmuls queue on PE ahead of the mix chain (PE is
            # in-order; DVE runs the softmax concurrently)
            pe0 = pp_e.tile([B, 2, HD], F32, name="pe")
            for k in range(KT):
                for c in range(2):
                    nc.tensor.matmul(
                        pe0[:, c, :], xt(k), wslice(0, k, c * HD, (c + 1) * HD),
                        start=(k == 0), stop=(k == KT - 1),
                    )

            # ---- top-2 + softmax per half -> dense mix coeffs [B, 64] ----
            mix_comb = _ctile(const, "mix_comb", [B, 2 * E], F32)
            for h in range(2):
                lh = logits[:, h * E : (h + 1) * E]
                mx1 = _ctile(const, f"mx1_{h}", [B, 1], F32)
                nc.vector.tensor_reduce(mx1[:], lh, axis=mybir.AxisListType.X, op=ALU.max)
                m1 = _ctile(const, f"m1_{h}", [B, E], F32)
                nc.vector.tensor_scalar(m1[:], lh, mx1[:], None, op0=ALU.is_ge)
                msk = _ctile(const, f"msk_{h}", [B, E], F32)
                nc.vector.scalar_tensor_tensor(
                    msk[:], m1[:], -1e30, lh, op0=ALU.mult, op1=ALU.add
                )
                mx2 = _ctile(const, f"mx2_{h}", [B, 1], F32)
                nc.vector.tensor_reduce(mx2[:], msk[:], axis=mybir.AxisListType.X, op=ALU.max)
                m2 = _ctile(const, f"m2_{h}", [B, E], F32)
                nc.vector.tensor_scalar(m2[:], msk[:], mx2[:], None, op0=ALU.is_ge)
                dgap = _ctile(const, f"dgap_{h}", [B, 1], F32)
                nc.vector.tensor_sub(dgap[:], mx2[:], mx1[:])
                ed = _ctile(const, f"ed_{h}", [B, 1], F32)
                nc.scalar.activation(ed[:], dgap[:], ACTF.Exp)
                den = _ctile(const, f"den_{h}", [B, 1], F32)
                nc.vector.tensor_scalar_add(den[:], ed[:], 1.0)
                p1 = _ctile(const, f"p1_{h}", [B, 1], F32)
                nc.vector.reciprocal(p1[:], den[:])
                p2 = _ctile(const, f"p2_{h}", [B, 1], F32)
                nc.vector.tensor_mul(p2[:], ed[:], p1[:])
                t2 = _ctile(const, f"t2_{h}", [B, E], F32)
                nc.vector.tensor_scalar_mul(t2[:], m2[:], p2[:])
                nc.vector.scalar_tensor_tensor(
                    mix_comb[:, h * E : (h + 1) * E], m1[:], p1[:], t2[:],
                    op0=ALU.mult, op1=ALU.add,
                )

            # ---- [B, 64] -> [64, B] transpose; bf16 for the sel matmuls ----
            ptm = pp_t.tile([2 * E, B], F32, name="pt")
            nc.tensor.transpose(ptm[:], mix_comb[:], ident[:])
            mixT = _ctile(const, "mixT", [2 * E, B], BF16)
            nc.vector.tensor_copy(mixT[:], ptm[:])

            # this core's weight/bias coefficients [B, 4+4]
            pml = pp_a.tile([B, 2 * E], F32, name="pa")
            nc.tensor.matmul(
                pml[:, 0 : 2 * EPC], mixT[:], sel, start=True, stop=True
            )
            mix_loc = _ctile(const, "mix_loc", [B, 2 * EPC], F32)
            nc.vector.tensor_copy(mix_loc[:], pml[:, 0 : 2 * EPC])

            # bias-mix transposed [4, B]: sel_b^T @ mixT (reuses mixT)
            pbt = pp_t.tile([2 * E, B], F32, name="pt")
            nc.tensor.matmul(
                pbt[0:EPC, :], sel[:, EPC : 2 * EPC], mixT[:], start=True, stop=True
            )
            mixbT = _ctile(const, "mixbT", [EPC, B], BF16)
            nc.vector.tensor_copy(mixbT[:], pbt[0:EPC, :])

            # ---- local bias term: mixb_loc @ bscl -> [B, D] ----
            pb = pp_e.tile([B, 2, HD], F32, name="pe")
            for c in range(2):
                nc.tensor.matmul(
                    pb[:, c, :], mixbT[:], bscl[:, c * HD : (c + 1) * HD],
                    start=True, stop=True,
                )
            bias_sb = _ctile(const, "bias_sb", [B, D], F32)
            for c in range(2):
                nc.vector.tensor_copy(bias_sb[:, c * HD : (c + 1) * HD], pb[:, c, :])

            # ---- experts: acc_e = (x @ W_e) * mix_loc[:, e] + acc_{e-1} ----
            prev = bias_sb
            for e in range(EPC - 1):
                if e == 0:
                    pe = pe0
                else:
                    pe = pp_e.tile([B, 2, HD], F32, name="pe")
                    for k in range(KT):
                        for c in range(2):
                            nc.tensor.matmul(
                                pe[:, c, :], xt(k), wslice(e, k, c * HD, (c + 1) * HD),
                                start=(k == 0), stop=(k == KT - 1),
                            )
                acc = _ctile(const, f"acc{e}", [B, D], F32)
                for c in range(2):
                    nc.vector.scalar_tensor_tensor(
                        acc[:, c * HD : (c + 1) * HD], pe[:, c, :], mix_loc[:, e : e + 1],
                        prev[:, c * HD : (c + 1) * HD], op0=ALU.mult, op1=ALU.add,
                    )
                prev = acc

            # last expert: per-half compute -> fold -> output DMA
            pel = pp_e.tile([B, 2, HD], F32, name="pe")
            accl = _ctile(const, f"acc{el}", [B, D], BF16)
            for h in range(2):
                hs, he = h * HD, (h + 1) * HD
                for k in range(KT):
                    nc.tensor.matmul(
                        pel[:, h, :], xt(k), wslice(el, k, hs, he),
                        start=(k == 0), stop=(k == KT - 1),
                    )
                if h == 0:
                    nc.vector.scalar_tensor_tensor(
                        accl[:, hs:he], pel[:, h, :], mix_loc[:, el : el + 1],
                        prev[:, hs:he], op0=ALU.mult, op1=ALU.add,
                    )
                    nc.scalar.dma_start(out_d[:, hs:he], accl[:, hs:he])
                else:
                    # final half in two 256-col pieces: fold of piece 1
                    # overlaps the output DMA of piece 0
                    for q in range(2):
                        qs, qe = hs + q * (HD // 2), hs + (q + 1) * (HD // 2)
                        nc.vector.scalar_tensor_tensor(
                            accl[:, qs:qe], pel[:, h, qs - hs : qe - hs],
                            mix_loc[:, el : el + 1],
                            prev[:, qs:qe], op0=ALU.mult, op1=ALU.add,
                        )
                        eng = nc.sync if q == 1 else nc.scalar
                        eng.dma_start(out_d[:, qs:qe], accl[:, qs:qe])

    nc.finalize()
    return nc


def make_input_maps(x, router_w, bias_router_w, expert_weights, expert_biases,
                    nf8=NF8):
    bf = ml_dtypes.bfloat16
    xt = np.asarray(x, dtype=np.float32).T.reshape(KT, 128, B).transpose(1, 0, 2)
    rw2 = (
        np.concatenate(
            [np.asarray(router_w, np.float32), np.asarray(bias_router_w, np.float32)],
            axis=1,
        )
        .reshape(KT, 128, 2 * E)
        .transpose(1, 0, 2)
    )
    xrw = np.ascontiguousarray(
        np.concatenate([xt, rw2], axis=2), dtype=np.float32
    )  # [128, KT, 192]

    ew = np.asarray(expert_weights, np.float32)
    eb = np.asarray(expert_biases, np.float32)

    in_maps = []
    for c in range(NCORES):
        selc = np.zeros((2 * E, 2 * EPC), dtype=np.float32)
        for j in range(EPC):
            selc[c * EPC + j, j] = 1.0
            selc[E + c * EPC + j, EPC + j] = 1.0
        selt = np.ascontiguousarray(selc.astype(bf))
        bsclt = np.ascontiguousarray(eb[c * EPC : (c + 1) * EPC].astype(bf))

        wall = (
            ew[c * EPC : (c + 1) * EPC]
            .reshape(EPC, KT, 128, D)
            .transpose(0, 2, 1, 3)
        )  # [EPC, 128, KT, D]
        wl = np.ascontiguousarray(wall[:, :, nf8:, :]).astype(bf)
        m = dict(xrw=xrw, selt=selt, bsclt=bsclt, wloc=wl)
        if nf8:
            m["wf8"] = np.ascontiguousarray(wall[:, :, 0:nf8, :] * 64.0).astype(
                ml_dtypes.float8_e4m3
            )
        in_maps.append(m)
    return in_maps


def kernel(x, router_w, bias_router_w, expert_weights, expert_biases, **bench_kwargs):
    in_maps = make_input_maps(x, router_w, bias_router_w, expert_weights, expert_biases)
    nc = build_program()
    res = run_bass_kernel_spmd(nc, in_maps, list(range(NCORES)), **bench_kwargs)
    out = np.zeros((B, D), dtype=np.float64)
    for r in res.results:
        out += r["out"].astype(np.float64)
    final = out.astype(np.float32)
    if bench_kwargs:
        kernel.last_result = res
    return final
